# revision 10
# baseline (speedup 1.0000x reference)
"""VQ codebook (vq_codebook) Trainium2 kernel.

Problem: z_e_x [32768, 256] f32, codebook weight [4096, 256] f32.
  distances[b,k] = ||z_b||^2 - 2 z_b.w_k + ||w_k||^2
  index = argmin_k distances           (first-tie)
  z_q_x = weight[index]
  sg_z_e_x == z_q_x numerically in forward.

Strategy (data-parallel over B, 8 cores, 4096 rows each):
  - argmin_k dist == argmax_k s where s[b,k] = z_b.w_k - ||w_k||^2/2
  - PE computes cross = zT.T @ wT per [128 rows x 2048 cols] PSUM tile
    (fp32, contraction over D=256 in two 128-blocks)
  - DVE tensor_tensor_reduce fuses the -e2/2 bias subtract with a
    running row-max (accum), writing biased scores to SBUF
  - DVE max_index recovers the argmax index (first occurrence)
  - two K-halves combined with a select; gpsimd indirect DMA gathers
    weight[index] rows from DRAM
"""

import numpy as np

import concourse.bass as bass
import concourse.bacc as bacc
import concourse.mybir as mybir
import concourse.tile as tile
from concourse.bass_utils import run_bass_kernel_spmd

B, K, D = 32768, 4096, 256
NCORES = 8
BS = B // NCORES            # rows per core
P = 128                     # partitions
NBLK = BS // P              # row blocks per core
NKH = 2                     # K halves (PSUM double buffering)
KH = K // NKH               # 2048 scores per half
MM_N = 512                  # fp32 moving-operand max / one PSUM bank
NEG_INF = -3.0e38

F32 = mybir.dt.float32
I32 = mybir.dt.int32
U32 = mybir.dt.uint32


def build_bass():
    nc = bacc.Bacc()
    zT = nc.dram_tensor("zT", [2, P, BS], F32, kind="ExternalInput")    # [dblk, d, b]
    wT = nc.dram_tensor("wT", [2, P, K], F32, kind="ExternalInput")     # [dblk, d, k]
    e2h = nc.dram_tensor("e2h", [P, K], F32, kind="ExternalInput")      # ||w_k||^2/2 replicated
    idx_out = nc.dram_tensor("idx_out", [P, NBLK], I32, kind="ExternalOutput")

    with tile.TileContext(nc) as tc:
        with (
            tc.tile_pool(name="resident", bufs=1) as res,
            tc.tile_pool(name="scores", bufs=2) as spool,
            tc.tile_pool(name="small", bufs=2) as small,
            tc.tile_pool(name="psum", bufs=2, space="PSUM") as psum,
        ):
            zT_sb = [res.tile([P, BS], F32, tag=f"zT{j}", name=f"zT_sb{j}") for j in range(2)]
            wT_sb = [res.tile([P, K], F32, tag=f"wT{j}", name=f"wT_sb{j}") for j in range(2)]
            e2h_sb = res.tile([P, K], F32, tag="e2h")
            # Load resident tiles via DMA into staging chunks, then funnel
            # through DVE copies: consumers then depend on the single DVE
            # semaphore instead of many DMA-queue semaphores (HW allows only
            # one sync-wait per instruction).
            LCH = 512  # load-chunk columns per dma_start
            loads = []
            for j in range(2):
                for c0 in range(0, BS, LCH):
                    loads.append((zT_sb[j], zT[j, :, :], c0))
                for c0 in range(0, K, LCH):
                    loads.append((wT_sb[j], wT[j, :, :], c0))
            for c0 in range(0, K, LCH):
                loads.append((e2h_sb, e2h[:, :], c0))
            for dst, src, c0 in loads:
                nc.sync.dma_start(dst[:, c0:c0 + LCH], src[:, c0:c0 + LCH])
                # in-place DVE copy: re-writes the chunk so downstream
                # consumers depend on the single DVE semaphore, not on the
                # many HW-DGE queue semaphores (1 sync-wait per inst limit)
                nc.vector.tensor_copy(dst[:, c0:c0 + LCH], dst[:, c0:c0 + LCH])

            # per-half row maxima and (fp32) indices for all blocks
            m_all = [res.tile([P, NBLK], F32, tag=f"m{h}", name=f"m_all{h}") for h in range(NKH)]
            if_all = [res.tile([P, NBLK], F32, tag=f"if{h}", name=f"if_all{h}") for h in range(NKH)]
            idx_all = res.tile([P, NBLK], I32, tag="idx_all")

            for blk in range(NBLK):
                bsl = slice(blk * P, (blk + 1) * P)
                for h in range(NKH):
                    ps = psum.tile([P, KH], F32, tag="ps")
                    for c in range(KH // MM_N):
                        k0 = c * MM_N
                        ksl = slice(h * KH + k0, h * KH + k0 + MM_N)
                        nc.tensor.matmul(
                            ps[:, k0:k0 + MM_N],
                            zT_sb[0][:, bsl],
                            wT_sb[0][:, ksl],
                            start=True, stop=False,
                        )
                        nc.tensor.matmul(
                            ps[:, k0:k0 + MM_N],
                            zT_sb[1][:, bsl],
                            wT_sb[1][:, ksl],
                            start=False, stop=True,
                        )
                    s_sb = spool.tile([P, KH], F32, tag="s")
                    nc.vector.tensor_sub(
                        s_sb[:], ps[:], e2h_sb[:, h * KH:(h + 1) * KH])
                    m8 = small.tile([P, 8], F32, tag="m8")
                    nc.vector.max(out=m8[:], in_=s_sb[:])
                    i8 = small.tile([P, 8], U32, tag="i8")
                    nc.vector.max_index(i8[:], m8[:], s_sb[:])
                    nc.vector.tensor_copy(m_all[h][:, blk:blk + 1], m8[:, 0:1])
                    nc.vector.tensor_copy(if_all[h][:, blk:blk + 1], i8[:, 0:1])

                # combine halves: idx = m0 >= m1 ? i0 : i1 + KH
                ge = small.tile([P, 1], U32, tag="ge")
                nc.vector.tensor_tensor(
                    out=ge[:], in0=m_all[0][:, blk:blk + 1],
                    in1=m_all[1][:, blk:blk + 1], op=mybir.AluOpType.is_ge,
                )
                i1p = small.tile([P, 1], F32, tag="i1p")
                nc.vector.tensor_scalar_add(
                    i1p[:], if_all[1][:, blk:blk + 1], float(KH))
                idxf = small.tile([P, 1], F32, tag="idxf")
                nc.vector.select(
                    idxf[:], ge[:], if_all[0][:, blk:blk + 1], i1p[:])
                idxi = small.tile([P, 1], I32, tag="idxi")
                nc.vector.tensor_copy(idxi[:], idxf[:])
                nc.vector.tensor_copy(idx_all[:, blk:blk + 1], idxi[:])

            nc.sync.dma_start(idx_out[:], idx_all[:])
    nc.finalize()
    return nc


_CACHE = {}


def _get_nc():
    if "nc" not in _CACHE:
        _CACHE["nc"] = build_bass()
    return _CACHE["nc"]


def make_in_maps(z_e_x: np.ndarray, weight: np.ndarray):
    z = np.ascontiguousarray(np.asarray(z_e_x, dtype=np.float32))
    w = np.ascontiguousarray(np.asarray(weight, dtype=np.float32))
    wT_full = np.ascontiguousarray(w.T.reshape(2, P, K))
    e2h_row = ((w.astype(np.float64) ** 2).sum(1) * 0.5).astype(np.float32)
    e2h_rep = np.ascontiguousarray(np.broadcast_to(e2h_row[None, :], (P, K)))
    in_maps = []
    for c in range(NCORES):
        zc = z[c * BS:(c + 1) * BS]
        zTc = np.ascontiguousarray(zc.T.reshape(2, P, BS))
        in_maps.append({"zT": zTc, "wT": wT_full, "e2h": e2h_rep})
    return in_maps


def kernel(z_e_x: np.ndarray, weight: np.ndarray):
    nc = _get_nc()
    in_maps = make_in_maps(z_e_x, weight)
    res = run_bass_kernel_spmd(nc, in_maps, list(range(NCORES))).results

    idx_full = np.empty(B, dtype=np.int32)
    for c in range(NCORES):
        idx_full[c * BS:(c + 1) * BS] = res[c]["idx_out"].T.reshape(-1)
    w = np.ascontiguousarray(np.asarray(weight, dtype=np.float32))
    zq_full = w[idx_full]
    sg = zq_full.copy()
    return sg, zq_full, idx_full
